# revision 22
# baseline (speedup 1.0000x reference)
"""Trainium2 Bass kernel for BatchLabelPropagation.

Per episode b (of 16), e=128 samples, c=512 channels:
  sq_dist = ||x_i - x_j||^2 / sqrt(c)                (pairwise, diag exactly 0)
  standardize sq_dist by GLOBAL (all-episode) masked mean/var (ddof=1)
  W = exp(-sq_dist), diag zeroed
  S = W * colscale_j,  colscale_j = 1/(1e-4 + rowsum(W)_j)
  P = inv(I - 0.2 S);  P rows L1-normalized;  out = log(P @ onehot + 1e-6)

Strategy: 8 NeuronCores, 2 episodes per core, pure data parallel, NO
collectives (a tiny AllReduce has a ~20us latency floor on TRN2). Two
launches with a tiny host-side stats combine between them:

  L1 (per core): xt (c-major) -> Gram G = X X^T on PE over 4 K-chunks.
     r = diag(G) = rowsum(G * (-eye/2)) on DVE; transposed to a row by a PE
     matmul against k*eye (k = -2/sqrt(c)) and broadcast down the
     partitions by a K=1 outer product with ones. sq = (G + rneg_i)*k + o
     needs no (1-eye) mask: the diagonal cancels EXACTLY in f32 because
     r_i is copied from G_ii (a - a/2 - a/2 == 0, and fl(-x*k) == -fl(x*k)).
     Local shifted one-pass stats (s = 2*sqrt(c), the analytic mean):
     per-row A = sum(sq) (DVE), Q = sum((sq-s)^2) (ACT Square+accum),
     packed with sq into a single (128, 260) output.

  host: A, Q summed (f64); var = (Q_off - D^2/cnt)/(cnt-1) with
     D = A - cnt*s, Q_off = Q - b*e*s^2; nis = -1/sqrt(var). 16 floats of
     glue - everything heavy stays on device.

  L2 (per core): W = exp(nis*sq) (one op for both episodes); diag zeroed
     via a (1-eye) mask then row-summed; the inverse is
     applied to B = [onehot | ones] by a Neumann series (||0.2 S|| ~ 0.17,
     4 terms reach the f32 floor): v <- B + W@(t*v), with B re-added inside
     PSUM via an identity-matmul seed. out = log(v[:,:5]/v[:,5] + 1e-6).
"""
import numpy as np

import concourse.bass as bass
import concourse.bacc as bacc
import concourse.tile as tile
from concourse import mybir
from concourse import bass_utils

NCORES = 8
B_FULL = 16
EP = B_FULL // NCORES  # episodes per core
E = 128
C = 512
KCHUNKS = C // 128
NCLASSES = 5
NB = NCLASSES + 1
SQW = EP * E  # sq columns in the packed L1 output
STW = SQW + 4  # + [A0, A1, Q0, Q1] stat columns

ALPHA = 0.2
EPS_OUT = 1e-6
EPS_DIAG = 1e-4
NEUMANN_ITERS = 4

SHIFT = float(2.0 * np.sqrt(np.float64(C)))  # analytic mean of sq_dist
CNT = float(B_FULL * E * (E - 1))
SQC = float(1.0 / np.sqrt(np.float64(C)))

F32 = mybir.dt.float32
AF = mybir.ActivationFunctionType
ALU = mybir.AluOpType
AX = mybir.AxisListType

_CACHE = {}


def _new_bacc(ncores):
    return bacc.Bacc(
        "TRN2",
        target_bir_lowering=False,
        debug=False,
        enable_asserts=True,
        num_devices=ncores,
    )


def _build_l1(ncores=NCORES):
    nc = _new_bacc(ncores)
    xt_d = nc.dram_tensor("xt", [EP, C, E], F32, kind="ExternalInput").ap()
    out_d = nc.dram_tensor("sqstat", [E, STW], F32, kind="ExternalOutput").ap()

    eyescale_np = (np.eye(E) * -0.5).astype(np.float32)
    eyk_np = (np.eye(E) * (-2.0 * SQC)).astype(np.float32)

    with tile.TileContext(nc) as tc:
        with (
            tc.tile_pool(name="sb", bufs=1) as sb,
            tc.tile_pool(name="scr", bufs=2) as scr,
            tc.tile_pool(name="ps", bufs=1, space="PSUM") as ps,
        ):
            # small consts first (memsets are cheap and dependency-free)
            ones_row = sb.tile([1, E], F32, tag="ones_row")
            nc.vector.memset(ones_row, 1.0)
            shift_col = sb.tile([E, 1], F32, tag="shift_col")
            nc.vector.memset(shift_col, -SHIFT)
            # dependency-free dummy activations pull the ACT table load to t=0
            dummy = sb.tile([1, 1], F32, tag="dummy")
            nc.scalar.activation(dummy, shift_col[0:1, 0:1], AF.Square)

            eyescale = sb.tile([E, E], F32, tag="eyescale")
            nc.gpsimd.dma_start(
                out=eyescale, in_=nc.inline_tensor(eyescale_np, name="c_eyescale").ap()
            )
            eyk = sb.tile([E, E], F32, tag="eyk")
            nc.gpsimd.dma_start(out=eyk, in_=nc.inline_tensor(eyk_np, name="c_eyk").ap())

            # x transposed; ep0 split in two DMAs so its Gram starts earlier
            h = KCHUNKS // 2
            xt0a = sb.tile([E, h, E], F32, tag="xt0a")
            xt0b = sb.tile([E, h, E], F32, tag="xt0b")
            xt_r = xt_d[0].rearrange("(k p) e -> p k e", p=E)
            nc.sync.dma_start(out=xt0a, in_=xt_r[:, 0:h, :])
            nc.sync.dma_start(out=xt0b, in_=xt_r[:, h:KCHUNKS, :])
            xt1 = sb.tile([E, KCHUNKS, E], F32, tag="xt1")
            nc.sync.dma_start(out=xt1, in_=xt_d[1].rearrange("(k p) e -> p k e", p=E))

            def xt_chunk(ep, k):
                if ep == 1:
                    return xt1[:, k, :]
                return (xt0a if k < h else xt0b)[:, k % h, :]

            out_sb = sb.tile([E, STW], F32, tag="out_sb")
            rneg = sb.tile([E, EP], F32, tag="rneg")

            g_ps = []
            for ep in range(EP):
                g = ps.tile([E, E], F32, tag=f"g{ep}")
                for k in range(KCHUNKS):
                    ck = xt_chunk(ep, k)
                    nc.tensor.matmul(g, ck, ck, start=(k == 0), stop=(k == KCHUNKS - 1))
                g_ps.append(g)
                # rneg = -r/2 = rowsum(G * (-eye/2))
                dscratch = scr.tile([E, E], F32, tag="dscratch")
                nc.vector.tensor_mul(dscratch, g, eyescale)
                nc.vector.tensor_reduce(
                    rneg[:, ep : ep + 1], dscratch, axis=AX.X, op=ALU.add
                )

            for ep in range(EP):
                # row of rneg*k via the scaled-eye matmul, broadcast down the
                # partitions with a K=1 outer product against ones
                rr_ps = ps.tile([1, E], F32, tag=f"rr{ep}")
                nc.tensor.matmul(rr_ps, rneg[:, ep : ep + 1], eyk, start=True, stop=True)
                rr = sb.tile([1, E], F32, tag=f"rrow{ep}")
                nc.scalar.copy(rr, rr_ps)
                o_ps = ps.tile([E, E], F32, tag=f"o{ep}")
                nc.tensor.matmul(o_ps, ones_row, rr, start=True, stop=True)

                # sq = (G + rneg_i)*k + o ; diagonal cancels exactly
                t1 = scr.tile([E, E], F32, tag="t1")
                nc.vector.tensor_scalar(
                    t1, g_ps[ep], rneg[:, ep : ep + 1], -2.0 * SQC,
                    op0=ALU.add, op1=ALU.mult,
                )
                sq_slice = out_sb[:, ep * E : (ep + 1) * E]
                nc.vector.tensor_add(sq_slice, t1, o_ps)
                nc.vector.tensor_reduce(
                    out_sb[:, SQW + ep : SQW + ep + 1], sq_slice, axis=AX.X, op=ALU.add
                )
                qscratch = scr.tile([E, E], F32, tag="qscratch")
                nc.scalar.activation(
                    qscratch, sq_slice, AF.Square,
                    bias=shift_col[:, 0:1],
                    accum_out=out_sb[:, SQW + 2 + ep : SQW + 3 + ep],
                )

            nc.sync.dma_start(out=out_d, in_=out_sb)

    nc.compile()
    return nc


def _build_l2(ncores=NCORES):
    nc = _new_bacc(ncores)
    sq_d = nc.dram_tensor("sqn", [E, 1 + SQW], F32, kind="ExternalInput").ap()
    bm_d = nc.dram_tensor("bmat", [EP, E, NB], F32, kind="ExternalInput").ap()
    out_d = nc.dram_tensor("out", [EP, E, NCLASSES], F32, kind="ExternalOutput").ap()

    mask01_np = (1.0 - np.eye(E, dtype=np.float32))
    eye_np = np.eye(E, dtype=np.float32)

    with tile.TileContext(nc) as tc:
        with (
            tc.tile_pool(name="sb", bufs=1) as sb,
            tc.tile_pool(name="ps", bufs=2, space="PSUM") as ps,
        ):
            lnbias_col = sb.tile([E, 1], F32, tag="lnbias_col")
            nc.vector.memset(lnbias_col, EPS_OUT)
            # dependency-free dummies pull the ACT table loads forward; Ln
            # first so the last (resident) set is the one exp needs
            dummy = sb.tile([1, 1], F32, tag="dummy")
            nc.scalar.activation(
                dummy, lnbias_col[0:1, 0:1], AF.Ln, bias=lnbias_col[0:1, 0:1]
            )
            dummy2 = sb.tile([1, 1], F32, tag="dummy2")
            nc.scalar.activation(dummy2, lnbias_col[0:1, 0:1], AF.Exp)

            mask01 = sb.tile([E, E], F32, tag="mask01")
            nc.gpsimd.dma_start(
                out=mask01, in_=nc.inline_tensor(mask01_np, name="c_mask01").ap()
            )
            bm = sb.tile([E, EP, NB], F32, tag="bm")
            nc.gpsimd.dma_start(out=bm, in_=bm_d.rearrange("ep i j -> i ep j"))

            # episode 0's DMA carries nis (host-replicated) in column 0
            sq0n = sb.tile([E, 1 + E], F32, tag="sq0n")
            nc.sync.dma_start(out=sq0n, in_=sq_d[:, 0 : 1 + E])
            nis_col = sq0n[:, 0:1]
            sq1 = sb.tile([E, E], F32, tag="sq1")
            nc.sync.dma_start(out=sq1, in_=sq_d[:, 1 + E : 1 + 2 * E])
            sq = [sq0n[:, 1 : 1 + E], sq1]
            eye = sb.tile([E, E], F32, tag="eye")
            nc.sync.dma_start(out=eye, in_=nc.inline_tensor(eye_np, name="c_eye").ap())

            wz = []
            ts = sb.tile([E, EP], F32, tag="ts")
            for ep in range(EP):
                w = sb.tile([E, E], F32, tag=f"w{ep}")
                nc.scalar.activation(w, sq[ep], AF.Exp, scale=nis_col[:, 0:1])
                wz_t = sb.tile([E, E], F32, tag=f"wz{ep}")
                nc.vector.tensor_mul(wz_t, w, mask01)
                wz.append(wz_t)
                dcol = sb.tile([E, 1], F32, tag=f"dcol{ep}")
                nc.vector.tensor_reduce(dcol, wz_t, axis=AX.X, op=ALU.add)
                dn = sb.tile([E, 1], F32, tag=f"dn{ep}")
                nc.vector.tensor_scalar_add(dn, dcol, EPS_DIAG)
                trec = sb.tile([E, 1], F32, tag=f"trec{ep}")
                nc.vector.reciprocal(trec, dn)
                nc.vector.tensor_scalar_mul(ts[:, ep : ep + 1], trec, ALPHA)

            outv = sb.tile([E, EP, NCLASSES], F32, tag="outv")
            for ep in range(EP):
                bslice = bm[:, ep, :]
                tslice = ts[:, ep : ep + 1]
                u = sb.tile([E, NB], F32, tag=f"u{ep}")
                nc.vector.tensor_scalar_mul(u, bslice, tslice)
                v_ps = None
                for it in range(NEUMANN_ITERS):
                    v_ps = ps.tile([E, NB], F32, tag=f"v{ep}")
                    nc.tensor.matmul(v_ps, eye, bslice, start=True, stop=False)
                    nc.tensor.matmul(v_ps, wz[ep], u, start=False, stop=True)
                    if it < NEUMANN_ITERS - 1:
                        u = sb.tile([E, NB], F32, tag=f"u{ep}")
                        nc.vector.tensor_scalar_mul(u, v_ps, tslice)
                recip_l1 = sb.tile([E, 1], F32, tag=f"rl1{ep}")
                nc.vector.reciprocal(recip_l1, v_ps[:, NCLASSES : NCLASSES + 1])
                # out = Ln(v * (1/l1) + 1e-6) in one ACT op (per-partition scale)
                nc.scalar.activation(
                    outv[:, ep, :], v_ps[:, 0:NCLASSES], AF.Ln,
                    bias=lnbias_col[:, 0:1], scale=recip_l1[:, 0:1],
                )
            nc.sync.dma_start(out=out_d.rearrange("ep i j -> i ep j"), in_=outv)

    nc.compile()
    return nc


def _get(name, builder):
    if name not in _CACHE:
        _CACHE[name] = builder()
    return _CACHE[name]


def _prepare_l1_in_maps(x):
    x = np.ascontiguousarray(np.asarray(x, dtype=np.float32))
    xt = np.ascontiguousarray(x.transpose(0, 2, 1))  # (b, c, e)
    return [
        {"xt": np.ascontiguousarray(xt[c * EP : (c + 1) * EP])} for c in range(NCORES)
    ]


def _host_combine(sqstat_list):
    st = np.stack([s[:, SQW:] for s in sqstat_list]).astype(np.float64)  # (cores,E,4)
    A = float(st[..., 0:EP].sum())
    Q = float(st[..., EP : 2 * EP].sum())
    q_off = Q - B_FULL * E * SHIFT * SHIFT
    d = A - CNT * SHIFT
    var = (q_off - d * d / CNT) / (CNT - 1.0)
    return np.float32(-1.0 / np.sqrt(var))


def _prepare_l2_in_maps(res1, labels, nis):
    labels = np.asarray(labels)
    bmat = np.zeros((B_FULL, E, NB), np.float32)
    bmat[..., NCLASSES] = 1.0
    for j in range(NCLASSES):
        bmat[..., j] = (labels == j).astype(np.float32)
    maps = []
    for c in range(NCORES):
        sqn = np.empty((E, 1 + SQW), np.float32)
        sqn[:, 0] = nis
        sqn[:, 1:] = res1[c]["sqstat"][:, 0:SQW]
        maps.append(
            {
                "sqn": sqn,
                "bmat": np.ascontiguousarray(bmat[c * EP : (c + 1) * EP]),
            }
        )
    return maps


def _run_spmd(nc, in_maps):
    """Run with retries: a crashed predecessor process can leave the
    accelerator in NRT_EXEC_UNIT_UNRECOVERABLE; it recovers on a fresh
    attempt after a short wait."""
    import time

    last = None
    for attempt in range(3):
        try:
            return bass_utils.run_bass_kernel_spmd(
                nc, in_maps, core_ids=list(range(NCORES))
            ).results
        except Exception as e:  # noqa: BLE001 - device transients are opaque
            last = e
            time.sleep(15 * (attempt + 1))
    raise last


def run(inputs):
    nc1 = _get("l1", _build_l1)
    nc2 = _get("l2", _build_l2)
    res1 = _run_spmd(nc1, _prepare_l1_in_maps(inputs["x"]))
    nis = _host_combine([r["sqstat"] for r in res1])
    res2 = _run_spmd(nc2, _prepare_l2_in_maps(res1, inputs["labels"], nis))
    out = np.concatenate([res2[c]["out"] for c in range(NCORES)], axis=0)
    return out.astype(np.float32)


def kernel(x, labels, nclasses):
    assert int(nclasses) == NCLASSES
    return run({"x": x, "labels": labels})


def timeline_estimate(trace_prefix=None):
    """Cost-model (TimelineSim) per-core estimates for both launches."""
    from concourse.timeline_sim import TimelineSim
    from trails.perfetto import LazyPerfetto

    for meth in ("enable_explicit_ordering", "reserve_process_order", "add_counter"):
        if not hasattr(LazyPerfetto, meth):
            setattr(LazyPerfetto, meth, lambda self, *a, **k: None)

    durs = []
    for name, builder in (("l1", _build_l1), ("l2", _build_l2)):
        nc = builder(ncores=1)
        trace = trace_prefix is not None
        tl = TimelineSim(nc, trace=trace)
        dur = tl.simulate()
        if trace and tl.perfetto is not None:
            tl.perfetto.save(f"{trace_prefix}_{name}.pftrace")
        durs.append(dur)
    return durs


if __name__ == "__main__":
    rng = np.random.default_rng(0)
    x = rng.standard_normal((B_FULL, E, C)).astype(np.float32)
    labels = rng.integers(0, NCLASSES + 1, size=(B_FULL, E))
    out = kernel(x, labels, NCLASSES)
    print("out", out.shape, out.dtype, out.min(), out.max())


# revision 25
# speedup vs baseline: 1.0017x; 1.0017x over previous
"""Trainium2 Bass kernel for BatchLabelPropagation.

Per episode b (of 16), e=128 samples, c=512 channels:
  sq_dist = ||x_i - x_j||^2 / sqrt(c)                (pairwise, diag exactly 0)
  standardize sq_dist by GLOBAL (all-episode) masked mean/var (ddof=1)
  W = exp(-sq_dist), diag zeroed
  S = W * colscale_j,  colscale_j = 1/(1e-4 + rowsum(W)_j)
  P = inv(I - 0.2 S);  P rows L1-normalized;  out = log(P @ onehot + 1e-6)

Strategy: 8 NeuronCores, 2 episodes per core, pure data parallel, NO
collectives (a tiny AllReduce has a ~20us latency floor on TRN2). Two
launches with a tiny host-side stats combine between them:

  L1 (per core): xt (c-major) -> Gram G = X X^T on PE over 4 K-chunks.
     r = diag(G) = rowsum(G * (-eye/2)) on DVE; transposed to a row by a PE
     matmul against k*eye (k = -2/sqrt(c)) and broadcast down the
     partitions by a K=1 outer product with ones. sq = (G + rneg_i)*k + o
     needs no (1-eye) mask: the diagonal cancels EXACTLY in f32 because
     r_i is copied from G_ii (a - a/2 - a/2 == 0, and fl(-x*k) == -fl(x*k)).
     Local shifted one-pass stats (s = 2*sqrt(c), the analytic mean):
     per-row A = sum(sq) (DVE), Q = sum((sq-s)^2) (ACT Square+accum),
     packed with sq into a single (128, 260) output.

  host: A, Q summed (f64); var = (Q_off - D^2/cnt)/(cnt-1) with
     D = A - cnt*s, Q_off = Q - b*e*s^2; nis = -1/sqrt(var). 16 floats of
     glue - everything heavy stays on device.

  L2 (per core): W = exp(nis*sq) (one op for both episodes); diag zeroed
     via a (1-eye) mask then row-summed; the inverse is
     applied to B = [onehot | ones] by a Neumann series (||0.2 S|| ~ 0.17,
     4 terms reach the f32 floor): v <- B + W@(t*v), with B re-added inside
     PSUM via an identity-matmul seed. out = log(v[:,:5]/v[:,5] + 1e-6).
"""
import numpy as np

import concourse.bass as bass
import concourse.bacc as bacc
import concourse.tile as tile
from concourse import mybir
from concourse import bass_utils

NCORES = 8
B_FULL = 16
EP = B_FULL // NCORES  # episodes per core
E = 128
C = 512
KCHUNKS = C // 128
NCLASSES = 5
NB = NCLASSES + 1
SQW = EP * E  # sq columns in the packed L1 output
STW = SQW + 4  # + [A0, A1, Q0, Q1] stat columns

ALPHA = 0.2
EPS_OUT = 1e-6
EPS_DIAG = 1e-4
NEUMANN_ITERS = 4

SHIFT = float(2.0 * np.sqrt(np.float64(C)))  # analytic mean of sq_dist
CNT = float(B_FULL * E * (E - 1))
SQC = float(1.0 / np.sqrt(np.float64(C)))

F32 = mybir.dt.float32
AF = mybir.ActivationFunctionType
ALU = mybir.AluOpType
AX = mybir.AxisListType

_CACHE = {}


def _new_bacc(ncores):
    return bacc.Bacc(
        "TRN2",
        target_bir_lowering=False,
        debug=False,
        enable_asserts=True,
        num_devices=ncores,
    )


def _build_l1(ncores=NCORES):
    nc = _new_bacc(ncores)
    xt_d = nc.dram_tensor("xt", [EP, C, E], F32, kind="ExternalInput").ap()
    out_d = nc.dram_tensor("sqstat", [E, STW], F32, kind="ExternalOutput").ap()

    eyescale_np = (np.eye(E) * -0.5).astype(np.float32)
    eyk_np = (np.eye(E) * (-2.0 * SQC)).astype(np.float32)

    with tile.TileContext(nc) as tc:
        with (
            tc.tile_pool(name="sb", bufs=1) as sb,
            tc.tile_pool(name="scr", bufs=2) as scr,
            tc.tile_pool(name="ps", bufs=1, space="PSUM") as ps,
        ):
            # small consts first (memsets are cheap and dependency-free)
            ones_row = sb.tile([1, E], F32, tag="ones_row")
            nc.vector.memset(ones_row, 1.0)
            shift_col = sb.tile([E, 1], F32, tag="shift_col")
            nc.vector.memset(shift_col, -SHIFT)
            # dependency-free dummy activations pull the ACT table load to t=0
            dummy = sb.tile([1, 1], F32, tag="dummy")
            nc.scalar.activation(dummy, shift_col[0:1, 0:1], AF.Square)

            eyescale = sb.tile([E, E], F32, tag="eyescale")
            nc.gpsimd.dma_start(
                out=eyescale, in_=nc.inline_tensor(eyescale_np, name="c_eyescale").ap()
            )
            eyk = sb.tile([E, E], F32, tag="eyk")
            nc.gpsimd.dma_start(out=eyk, in_=nc.inline_tensor(eyk_np, name="c_eyk").ap())

            # x transposed; ep0 split in two DMAs so its Gram starts earlier
            h = KCHUNKS // 2
            xt0a = sb.tile([E, h, E], F32, tag="xt0a")
            xt0b = sb.tile([E, h, E], F32, tag="xt0b")
            xt_r = xt_d[0].rearrange("(k p) e -> p k e", p=E)
            nc.sync.dma_start(out=xt0a, in_=xt_r[:, 0:h, :])
            nc.sync.dma_start(out=xt0b, in_=xt_r[:, h:KCHUNKS, :])
            xt1 = sb.tile([E, KCHUNKS, E], F32, tag="xt1")
            nc.sync.dma_start(out=xt1, in_=xt_d[1].rearrange("(k p) e -> p k e", p=E))

            def xt_chunk(ep, k):
                if ep == 1:
                    return xt1[:, k, :]
                return (xt0a if k < h else xt0b)[:, k % h, :]

            out_sb = sb.tile([E, STW], F32, tag="out_sb")
            rneg = sb.tile([E, EP], F32, tag="rneg")

            g_ps = []
            for ep in range(EP):
                g = ps.tile([E, E], F32, tag=f"g{ep}")
                for k in range(KCHUNKS):
                    ck = xt_chunk(ep, k)
                    nc.tensor.matmul(g, ck, ck, start=(k == 0), stop=(k == KCHUNKS - 1))
                g_ps.append(g)
                # rneg = -r/2 = rowsum(G * (-eye/2))
                dscratch = scr.tile([E, E], F32, tag="dscratch")
                nc.vector.tensor_mul(dscratch, g, eyescale)
                nc.vector.tensor_reduce(
                    rneg[:, ep : ep + 1], dscratch, axis=AX.X, op=ALU.add
                )

            for ep in range(EP):
                # row of rneg*k via the scaled-eye matmul, broadcast down the
                # partitions with a K=1 outer product against ones
                rr_ps = ps.tile([1, E], F32, tag=f"rr{ep}")
                nc.tensor.matmul(rr_ps, rneg[:, ep : ep + 1], eyk, start=True, stop=True)
                rr = sb.tile([1, E], F32, tag=f"rrow{ep}")
                nc.scalar.copy(rr, rr_ps)
                o_ps = ps.tile([E, E], F32, tag=f"o{ep}")
                nc.tensor.matmul(o_ps, ones_row, rr, start=True, stop=True)

                # sq = (G + rneg_i)*k + o ; diagonal cancels exactly
                t1 = scr.tile([E, E], F32, tag="t1")
                nc.vector.tensor_scalar(
                    t1, g_ps[ep], rneg[:, ep : ep + 1], -2.0 * SQC,
                    op0=ALU.add, op1=ALU.mult,
                )
                sq_slice = out_sb[:, ep * E : (ep + 1) * E]
                nc.vector.tensor_add(sq_slice, t1, o_ps)
                nc.vector.tensor_reduce(
                    out_sb[:, SQW + ep : SQW + ep + 1], sq_slice, axis=AX.X, op=ALU.add
                )
                qscratch = scr.tile([E, E], F32, tag="qscratch")
                nc.scalar.activation(
                    qscratch, sq_slice, AF.Square,
                    bias=shift_col[:, 0:1],
                    accum_out=out_sb[:, SQW + 2 + ep : SQW + 3 + ep],
                )

            nc.sync.dma_start(out=out_d, in_=out_sb)

    nc.compile()
    return nc


def _build_l2(ncores=NCORES):
    nc = _new_bacc(ncores)
    sq_d = nc.dram_tensor("sqn", [E, 1 + SQW], F32, kind="ExternalInput").ap()
    bm_d = nc.dram_tensor("bmat", [EP, E, NB], F32, kind="ExternalInput").ap()
    out_d = nc.dram_tensor("out", [EP, E, NCLASSES], F32, kind="ExternalOutput").ap()

    mask01_np = (1.0 - np.eye(E, dtype=np.float32))
    eye_np = np.eye(E, dtype=np.float32)

    with tile.TileContext(nc) as tc:
        with (
            tc.tile_pool(name="sb", bufs=1) as sb,
            tc.tile_pool(name="ps", bufs=2, space="PSUM") as ps,
        ):
            lnbias_col = sb.tile([E, 1], F32, tag="lnbias_col")
            nc.vector.memset(lnbias_col, EPS_OUT)
            # dependency-free dummies pull the ACT table loads forward; Ln
            # first so the last (resident) set is the one exp needs
            dummy = sb.tile([1, 1], F32, tag="dummy")
            nc.scalar.activation(
                dummy, lnbias_col[0:1, 0:1], AF.Ln, bias=lnbias_col[0:1, 0:1]
            )
            dummy2 = sb.tile([1, 1], F32, tag="dummy2")
            nc.scalar.activation(dummy2, lnbias_col[0:1, 0:1], AF.Exp)

            mask01 = sb.tile([E, E], F32, tag="mask01")
            nc.gpsimd.dma_start(
                out=mask01, in_=nc.inline_tensor(mask01_np, name="c_mask01").ap()
            )
            bm = sb.tile([E, EP, NB], F32, tag="bm")
            nc.gpsimd.dma_start(out=bm, in_=bm_d.rearrange("ep i j -> i ep j"))

            # episode 0's DMA carries nis (host-replicated) in column 0
            sq0n = sb.tile([E, 1 + E], F32, tag="sq0n")
            nc.sync.dma_start(out=sq0n, in_=sq_d[:, 0 : 1 + E])
            nis_col = sq0n[:, 0:1]
            sq1 = sb.tile([E, E], F32, tag="sq1")
            nc.sync.dma_start(out=sq1, in_=sq_d[:, 1 + E : 1 + 2 * E])
            sq = [sq0n[:, 1 : 1 + E], sq1]
            eye = sb.tile([E, E], F32, tag="eye")
            nc.sync.dma_start(out=eye, in_=nc.inline_tensor(eye_np, name="c_eye").ap())

            wz = []
            ts = sb.tile([E, EP], F32, tag="ts")
            for ep in range(EP):
                w = sb.tile([E, E], F32, tag=f"w{ep}")
                nc.scalar.activation(w, sq[ep], AF.Exp, scale=nis_col[:, 0:1])
                wz_t = sb.tile([E, E], F32, tag=f"wz{ep}")
                nc.vector.tensor_mul(wz_t, w, mask01)
                wz.append(wz_t)
                dcol = sb.tile([E, 1], F32, tag=f"dcol{ep}")
                nc.vector.tensor_reduce(dcol, wz_t, axis=AX.X, op=ALU.add)
                # t = alpha/(1e-4+d) == 1/((d + 1e-4)/alpha): one fused
                # scale-and-bias op, then the reciprocal lands on t directly
                dn = sb.tile([E, 1], F32, tag=f"dn{ep}")
                nc.vector.tensor_scalar(
                    dn, dcol, 1.0 / ALPHA, EPS_DIAG / ALPHA,
                    op0=ALU.mult, op1=ALU.add,
                )
                nc.vector.reciprocal(ts[:, ep : ep + 1], dn)

            outv = sb.tile([E, EP, NCLASSES], F32, tag="outv")
            for ep in range(EP):
                bslice = bm[:, ep, :]
                tslice = ts[:, ep : ep + 1]
                u = sb.tile([E, NB], F32, tag=f"u{ep}")
                nc.vector.tensor_scalar_mul(u, bslice, tslice)
                v_ps = None
                for it in range(NEUMANN_ITERS):
                    v_ps = ps.tile([E, NB], F32, tag=f"v{ep}")
                    nc.tensor.matmul(v_ps, eye, bslice, start=True, stop=False)
                    nc.tensor.matmul(v_ps, wz[ep], u, start=False, stop=True)
                    if it < NEUMANN_ITERS - 1:
                        u = sb.tile([E, NB], F32, tag=f"u{ep}")
                        nc.vector.tensor_scalar_mul(u, v_ps, tslice)
                recip_l1 = sb.tile([E, 1], F32, tag=f"rl1{ep}")
                nc.vector.reciprocal(recip_l1, v_ps[:, NCLASSES : NCLASSES + 1])
                # out = Ln(v * (1/l1) + 1e-6) in one ACT op (per-partition scale)
                nc.scalar.activation(
                    outv[:, ep, :], v_ps[:, 0:NCLASSES], AF.Ln,
                    bias=lnbias_col[:, 0:1], scale=recip_l1[:, 0:1],
                )
            nc.sync.dma_start(out=out_d.rearrange("ep i j -> i ep j"), in_=outv)

    nc.compile()
    return nc


def _get(name, builder):
    if name not in _CACHE:
        _CACHE[name] = builder()
    return _CACHE[name]


def _prepare_l1_in_maps(x):
    x = np.ascontiguousarray(np.asarray(x, dtype=np.float32))
    xt = np.ascontiguousarray(x.transpose(0, 2, 1))  # (b, c, e)
    return [
        {"xt": np.ascontiguousarray(xt[c * EP : (c + 1) * EP])} for c in range(NCORES)
    ]


def _host_combine(sqstat_list):
    st = np.stack([s[:, SQW:] for s in sqstat_list]).astype(np.float64)  # (cores,E,4)
    A = float(st[..., 0:EP].sum())
    Q = float(st[..., EP : 2 * EP].sum())
    q_off = Q - B_FULL * E * SHIFT * SHIFT
    d = A - CNT * SHIFT
    var = (q_off - d * d / CNT) / (CNT - 1.0)
    return np.float32(-1.0 / np.sqrt(var))


def _prepare_l2_in_maps(res1, labels, nis):
    labels = np.asarray(labels)
    bmat = np.zeros((B_FULL, E, NB), np.float32)
    bmat[..., NCLASSES] = 1.0
    for j in range(NCLASSES):
        bmat[..., j] = (labels == j).astype(np.float32)
    maps = []
    for c in range(NCORES):
        sqn = np.empty((E, 1 + SQW), np.float32)
        sqn[:, 0] = nis
        sqn[:, 1:] = res1[c]["sqstat"][:, 0:SQW]
        maps.append(
            {
                "sqn": sqn,
                "bmat": np.ascontiguousarray(bmat[c * EP : (c + 1) * EP]),
            }
        )
    return maps


def _run_spmd(nc, in_maps):
    """Run with retries: a crashed predecessor process can leave the
    accelerator in NRT_EXEC_UNIT_UNRECOVERABLE; it recovers on a fresh
    attempt after a short wait."""
    import time

    last = None
    for attempt in range(3):
        try:
            return bass_utils.run_bass_kernel_spmd(
                nc, in_maps, core_ids=list(range(NCORES))
            ).results
        except Exception as e:  # noqa: BLE001 - device transients are opaque
            last = e
            time.sleep(15 * (attempt + 1))
    raise last


def run(inputs):
    nc1 = _get("l1", _build_l1)
    nc2 = _get("l2", _build_l2)
    res1 = _run_spmd(nc1, _prepare_l1_in_maps(inputs["x"]))
    nis = _host_combine([r["sqstat"] for r in res1])
    res2 = _run_spmd(nc2, _prepare_l2_in_maps(res1, inputs["labels"], nis))
    out = np.concatenate([res2[c]["out"] for c in range(NCORES)], axis=0)
    return out.astype(np.float32)


def kernel(x, labels, nclasses):
    assert int(nclasses) == NCLASSES
    return run({"x": x, "labels": labels})


def timeline_estimate(trace_prefix=None):
    """Cost-model (TimelineSim) per-core estimates for both launches."""
    from concourse.timeline_sim import TimelineSim
    from trails.perfetto import LazyPerfetto

    for meth in ("enable_explicit_ordering", "reserve_process_order", "add_counter"):
        if not hasattr(LazyPerfetto, meth):
            setattr(LazyPerfetto, meth, lambda self, *a, **k: None)

    durs = []
    for name, builder in (("l1", _build_l1), ("l2", _build_l2)):
        nc = builder(ncores=1)
        trace = trace_prefix is not None
        tl = TimelineSim(nc, trace=trace)
        dur = tl.simulate()
        if trace and tl.perfetto is not None:
            tl.perfetto.save(f"{trace_prefix}_{name}.pftrace")
        durs.append(dur)
    return durs


if __name__ == "__main__":
    rng = np.random.default_rng(0)
    x = rng.standard_normal((B_FULL, E, C)).astype(np.float32)
    labels = rng.integers(0, NCLASSES + 1, size=(B_FULL, E))
    out = kernel(x, labels, NCLASSES)
    print("out", out.shape, out.dtype, out.min(), out.max())
